# revision 29
# baseline (speedup 1.0000x reference)
"""3-layer GAT (graph attention network) on 8 Trainium2 NeuronCores.

Problem: N=4096 nodes, dense adjacency [N,N], 3 GAT layers
  (128 -> 4x64, 256 -> 4x64, 256 -> 1x64), LeakyReLU(0.2) attention,
  masked softmax, ELU between layers.

Sharding: 1D row partition of the attention matrix. Each core owns
IB=512 rows i (queries). Scores/softmax/aggregation for those rows are
computed locally in transposed layout P[j, i] (j on partitions) so the
aggregation matmul contracts j on the partition axis with no transposes.

Score trick (the big win vs a direct port): with e = el_i + er_j,
  exp(leakyrelu(e)) = max(exp(e), exp(0.2 e))
                    = exp(el_i) * max(exp(er_j), exp(-0.8 el_i)*exp(0.2 er_j))
and the exp(el_i) row factor cancels in the softmax, so the masked
unnormalized weight is
  p'[j, i] = mask * max(B_j, F_i * D_j),
  B = exp(er), D = exp(0.2 er), F = exp(-0.8 el).
B/D live as per-partition scalars ([128, NT] transposed via PE), F as a
broadcast tile [128, IB]. Each j-tile then costs ONE fused VectorE
tensor_scalar (bf16 4x: t = max(F*D_j, B_j)) plus a share of ONE batched
mask tensor_tensor (bf16 2x). No per-element ScalarE work and no PE
score matmuls at all.

Inner loop per (head, j-tile):
  t = max(F*D_j, B_j)        -- VectorE tensor_scalar, bf16 4x
  p = t * adjT mask          -- VectorE tensor_mul, bf16 2x, batched x4
  oT[o,i] += [h|1]^T @ p     -- TensorE, accumulating; row 64 = denom

Softmax denominators are reciprocated in transposed layout ([128, 4*H]
after PE transposes) so the DVE divide iterates over 4 elements/lane
instead of 512.

kernel(**inputs) takes the full unsharded inputs and returns the full
[4096, 64] output.
"""

import numpy as np
import ml_dtypes

import concourse.bass as bass
import concourse.mybir as mybir
import concourse.tile as tile
from concourse import bacc
from concourse.bass_utils import run_bass_kernel_spmd

F32 = mybir.dt.float32
BF16 = mybir.dt.bfloat16
F32R = mybir.dt.float32r
AF = mybir.ActivationFunctionType
ALU = mybir.AluOpType

NC = 8          # cores
N = 4096        # nodes
NT = N // 128   # 32 j-tiles
GJ = 4          # j-tiles per mask-multiply batch
IB = N // NC    # 512 rows per core
H = 4           # heads (layers 0,1)
O = 64          # per-head output dim
D0 = 128        # layer-0 input dim
D1 = H * O      # 256, layer-1/2 input dim
ALPHA = 0.2

_CACHE = {}


def _dma(nc, out, in_):
    nc.sync.dma_start(out=out, in_=in_)


def _build(sim_mode=False, reps=1, debug=False):
    nc = bacc.Bacc(None, target_bir_lowering=False,
                   num_devices=1 if sim_mode else NC)

    xTf = nc.dram_tensor("xTf", [D0, N], F32, kind="ExternalInput")
    xT0 = nc.dram_tensor("xT0", [D0, IB], F32, kind="ExternalInput")
    maskT = nc.dram_tensor("maskT", [NT, 128, IB], BF16, kind="ExternalInput")
    w0 = nc.dram_tensor("w0", [H, D0, O], F32, kind="ExternalInput")
    w1 = nc.dram_tensor("w1", [H, D1, O], F32, kind="ExternalInput")
    w2 = nc.dram_tensor("w2", [1, D1, O], F32, kind="ExternalInput")
    wlr0 = nc.dram_tensor("wlr0", [D0, 2 * H], F32, kind="ExternalInput")
    wlr1 = nc.dram_tensor("wlr1", [D1, 2 * H], F32, kind="ExternalInput")
    wlr2 = nc.dram_tensor("wlr2", [D1, 2], F32, kind="ExternalInput")
    y = nc.dram_tensor("y", [O, IB], F32, kind="ExternalOutput")
    if debug:
        dbg = {
            "d_BT0": nc.dram_tensor("d_BT0", [128, NT], F32, kind="ExternalOutput"),
            "d_DT0": nc.dram_tensor("d_DT0", [128, NT], F32, kind="ExternalOutput"),
            "d_Fbc0": nc.dram_tensor("d_Fbc0", [128, IB], F32, kind="ExternalOutput"),
            "d_pg00": nc.dram_tensor("d_pg00", [128, GJ * IB], F32, kind="ExternalOutput"),
            "d_oT0": nc.dram_tensor("d_oT0", [65, IB], F32, kind="ExternalOutput"),
            "d_z0": nc.dram_tensor("d_z0", [O, IB], F32, kind="ExternalOutput"),
            "d_hall0": nc.dram_tensor("d_hall0", [128, H * 65], F32, kind="ExternalOutput"),
            "d_BT1": nc.dram_tensor("d_BT1", [128, NT], F32, kind="ExternalOutput"),
            "d_DT1": nc.dram_tensor("d_DT1", [128, NT], F32, kind="ExternalOutput"),
            "d_Fbc1": nc.dram_tensor("d_Fbc1", [128, IB], F32, kind="ExternalOutput"),
            "d_xTn0": nc.dram_tensor("d_xTn0", [128, IB], F32, kind="ExternalOutput"),
            "d_hall1": nc.dram_tensor("d_hall1", [128, H * 65], F32, kind="ExternalOutput"),
            "d_oT1": nc.dram_tensor("d_oT1", [65, IB], F32, kind="ExternalOutput"),
            "d_z1": nc.dram_tensor("d_z1", [O, IB], F32, kind="ExternalOutput"),
            "d_gersb": nc.dram_tensor("d_gersb", [NC * H, IB], F32, kind="ExternalOutput"),
        }

    rg = [list(range(NC))]

    with tile.TileContext(nc) as tc:
        with (
            tc.tile_pool(name="const", bufs=1) as cpool,
            tc.tile_pool(name="work", bufs=2) as wpool,
            tc.tile_pool(name="psum", bufs=2, space="PSUM") as pp,
            tc.tile_pool(name="dram", bufs=1, space="DRAM") as dpool,
        ):
            # ---------- constants / resident tiles ----------
            mask_sb = cpool.tile([128, NT * IB], BF16)
            mv = mask_sb[:].rearrange("p (t i) -> p t i", t=NT)
            for g in range(8):
                _dma(nc, mv[:, g * 4:(g + 1) * 4, :],
                     maskT[g * 4:(g + 1) * 4].rearrange("t p i -> p t i"))

            xTf_sb = cpool.tile([D0, N], F32)
            _dma(nc, xTf_sb[:], xTf[:])
            xT0_sb = cpool.tile([D0, IB], F32)
            _dma(nc, xT0_sb[:], xT0[:])

            w0_sb = cpool.tile([D0, H * O], F32)
            _dma(nc, w0_sb[:].rearrange("d (h o) -> d h o", h=H),
                 w0[:].rearrange("h d o -> d h o"))
            w1_sb = cpool.tile([128, 2 * H * O], F32)  # [kc] chunks side by side
            w1v = w1_sb[:].rearrange("d (k h o) -> d k h o", k=2, h=H)
            w1s = w1[:].rearrange("h (k d) o -> k d h o", k=2)
            for kc in range(2):
                _dma(nc, w1v[:, kc], w1s[kc])
            w2_sb = cpool.tile([128, 2 * O], F32)
            w2v = w2_sb[:].rearrange("d (k h o) -> d k h o", k=2, h=1)
            w2s = w2[:].rearrange("h (k d) o -> k d h o", k=2)
            for kc in range(2):
                _dma(nc, w2v[:, kc], w2s[kc])

            wlr0_sb = cpool.tile([D0, 2 * H], F32)
            _dma(nc, wlr0_sb[:], wlr0[:])
            wlr1_sb = cpool.tile([128, 2 * 2 * H], F32)
            wlr1v = wlr1_sb[:].rearrange("d (k c) -> d k c", k=2)
            _dma(nc, wlr1v, wlr1[:].rearrange("(k d) c -> d k c", k=2))
            wlr2_sb = cpool.tile([128, 2 * 2], F32)
            wlr2v = wlr2_sb[:].rearrange("d (k c) -> d k c", k=2)
            _dma(nc, wlr2v, wlr2[:].rearrange("(k d) c -> d k c", k=2))

            xTf_bf = cpool.tile([D0, N], BF16)
            nc.vector.tensor_copy(xTf_bf[:], xTf_sb[:])
            w0_bf = cpool.tile([D0, H * O], BF16)
            nc.vector.tensor_copy(w0_bf[:], w0_sb[:])
            w1_bf = cpool.tile([128, 2 * H * O], BF16)
            nc.vector.tensor_copy(w1_bf[:], w1_sb[:])
            w2_bf = cpool.tile([128, 2 * O], BF16)
            nc.vector.tensor_copy(w2_bf[:], w2_sb[:])

            # fp32r-rounded copies for the next-layer score-term matmuls
            # (the BIR verifier requires f32r matmul operands to be
            # produced rounded; layer-0 uses plain f32 matmuls instead)
            wlr1_r = cpool.tile([128, 2 * 2 * H], F32R)
            nc.vector.tensor_copy(wlr1_r[:], wlr1_sb[:])
            wlr2_r = cpool.tile([128, 2 * 2], F32R)
            nc.vector.tensor_copy(wlr2_r[:], wlr2_sb[:])

            ones_c = cpool.tile([1, IB], F32R)
            nc.vector.memset(ones_c[:].bitcast(F32), 1.0)
            ones128 = cpool.tile([128, 128], F32)
            nc.vector.memset(ones128[:], 1.0)
            id_sb = cpool.tile([128, 128], F32)
            nc.gpsimd.affine_select(id_sb[:], ones128[:], [[-1, 128]],
                                    ALU.is_equal, 0.0, base=0,
                                    channel_multiplier=1)

            def bd_tiles(nheads, lname):
                BT = [wpool.tile([128, NT], F32, tag=f"BT{h}", bufs=2,
                                 name=f"BT_{lname}_{h}") for h in range(nheads)]
                DT = [wpool.tile([128, NT], F32, tag=f"DT{h}", bufs=2,
                                 name=f"DT_{lname}_{h}") for h in range(nheads)]
                return BT, DT

            def fbc_tiles(nheads, pelr, lname):
                """rows 0:nheads of pelr (el) -> per-head F = exp(-0.8 el)
                broadcast tiles [128, IB] bf16 via ones-outer-product.
                (engine APs must start at partition 0/32/64, so the per-head
                rows are staged to partition 0 with tiny SBUF DMAs)"""
                F_all = wpool.tile([nheads, IB], F32R, tag="Fall", bufs=1,
                                   name=f"Fall_{lname}")
                nc.scalar.activation(F_all[:], pelr[0:nheads, :], AF.Exp,
                                     scale=-0.8)
                F_bc = []
                for h in range(nheads):
                    r = wpool.tile([1, IB], F32R, tag="Frow", bufs=2,
                                   name=f"Frow_{lname}_{h}")
                    _dma(nc, r[:], F_all[h:h + 1, :])
                    fb = pp.tile([128, IB], F32, tag="work",
                                 name=f"fb_{lname}_{h}")
                    nc.tensor.matmul(fb[:], ones_c[0:1, 0:128], r[0:1, :])
                    t = wpool.tile([128, IB], BF16, tag=f"Fbc{h}", bufs=1,
                                   name=f"Fbc_{lname}_{h}")
                    nc.scalar.copy(t[:], fb[:])
                    F_bc.append(t)
                return F_bc

            for rep in range(reps):
                # ---------- DRAM bounce buffers for collectives ----------
                gh1_in = dpool.tile([IB, D1], BF16)
                gh1 = dpool.tile([N, D1], BF16, addr_space="Shared")
                ger1_in = dpool.tile([H, IB], F32)
                ger1 = dpool.tile([NC * H, IB], F32, addr_space="Shared")
                gh2_in = dpool.tile([IB, O], BF16)
                gh2 = dpool.tile([N, O], BF16, addr_space="Shared")
                ger2_in = dpool.tile([1, IB], F32)
                ger2 = dpool.tile([NC, IB], F32, addr_space="Shared")

                # ================= layer 0 prep =================
                # er0 for ALL nodes, directly transposed: per j-block
                # erT0[:, t*2H:(t+1)*2H] = x[t-block]^T @ wlr0  ([128, 2H])
                erT0 = pp.tile([128, NT * 2 * H], F32, tag="erT", bufs=1,
                               name=f"erT0_{rep}")
                for t in range(NT):
                    nc.tensor.matmul(
                        erT0[:, t * 2 * H:(t + 1) * 2 * H],
                        xTf_sb[:, t * 128:(t + 1) * 128],
                        wlr0_sb[:])
                erT0v = erT0[:].rearrange("p (t c) -> p c t", c=2 * H)
                BT0, DT0 = bd_tiles(H, f"r{rep}l0")
                for h in range(H):
                    nc.scalar.activation(BT0[h][:], erT0v[:, H + h], AF.Exp)
                    nc.scalar.activation(DT0[h][:], erT0v[:, H + h], AF.Exp,
                                         scale=0.2)
                # el0 (local rows) -> F rows -> broadcast tiles
                pl0 = pp.tile([2 * H, IB], F32, tag="work", name=f"pl0_{rep}")
                nc.tensor.matmul(pl0[:], wlr0_sb[:], xT0_sb[:])
                F_bc0 = fbc_tiles(H, pl0, f"r{rep}l0")

                if debug:
                    _dma(nc, dbg["d_BT0"][:], BT0[0][:])
                    _dma(nc, dbg["d_DT0"][:], DT0[0][:])
                    tf_ = wpool.tile([128, IB], F32, tag="dbg3", bufs=1)
                    nc.vector.tensor_copy(tf_[:], F_bc0[0][:])
                    _dma(nc, dbg["d_Fbc0"][:], tf_[:])

                # full h0 (redundantly per core) -> h_all0 [128, NT*(H*65)]
                h_all0 = wpool.tile([128, NT * H * 65], BF16, tag="h_all",
                                    bufs=2)
                h0v = h_all0[:].rearrange("p (t h c) -> p t h c", t=NT, h=H)
                h0ones = h_all0[:].rearrange("p (q c) -> p q c", c=65)
                nc.gpsimd.memset(h0ones[:, :, O:65], 1.0)
                for jt in range(NT):
                    ph = pp.tile([128, H * O], F32, tag="work",
                                 name=f"ph0_{rep}_{jt}")
                    for h in range(H):
                        nc.tensor.matmul(
                            ph[:, h * O:(h + 1) * O],
                            xTf_bf[:, jt * 128:(jt + 1) * 128],
                            w0_bf[:, h * O:(h + 1) * O])
                    nc.scalar.copy(h0v[:, jt, :, 0:O],
                                   ph[:].rearrange("p (h o) -> p h o", h=H))

                if debug:
                    th_ = wpool.tile([128, H * 65], F32, tag="dbg4", bufs=1)
                    nc.vector.tensor_copy(th_[:], h_all0[:, 0:H * 65])
                    _dma(nc, dbg["d_hall0"][:], th_[:])

                def attention(nheads, h_all, F_bc, BT, DT, lname,
                              on_head_done=None):
                    """Row-block attention; oT[h] [65, IB] PSUM accumulators
                    (row 64 = softmax denominator). on_head_done(h, oT_h) is
                    emitted two score-groups into the NEXT head so the
                    engines never stall on the accumulation tail."""
                    oT = [pp.tile([65, IB], F32, tag=f"oT{h}", bufs=1,
                                  name=f"oT_{lname}_{h}")
                          for h in range(nheads)]
                    POOLG = 3   # per-head group offloaded to idle GPSIMD
                    for h in range(nheads):
                        lag = None
                        for g in range(NT // GJ):
                            use_pool = (g == POOLG)
                            eng = nc.gpsimd if use_pool else nc.vector
                            sg = wpool.tile([128, GJ * IB], BF16,
                                            tag="sgp" if use_pool else "sg",
                                            bufs=1 if use_pool else 2,
                                            name=f"s_{lname}_{h}_{g}")
                            for jj in range(GJ):
                                jt = g * GJ + jj
                                eng.tensor_scalar(
                                    sg[:, jj * IB:(jj + 1) * IB],
                                    F_bc[h][:],
                                    DT[h][:, jt:jt + 1],
                                    BT[h][:, jt:jt + 1],
                                    ALU.mult, ALU.max)
                            pg = wpool.tile([128, GJ * IB], BF16,
                                            tag="pgp" if use_pool else "pg",
                                            bufs=1 if use_pool else 2,
                                            name=f"p_{lname}_{h}_{g}")
                            eng.tensor_mul(
                                pg[:], sg[:],
                                mask_sb[:, g * GJ * IB:(g + 1) * GJ * IB])
                            if use_pool:
                                # defer this group's aggregation until the
                                # slow GPSIMD op has had time to finish
                                lag = (g, pg)
                                continue
                            if debug and lname.endswith("l0") and h == 0 and g == 0:
                                t_ = wpool.tile([128, GJ * IB], F32, tag="dbg1", bufs=1)
                                nc.vector.tensor_copy(t_[:], pg[:])
                                _dma(nc, dbg["d_pg00"][:], t_[:])
                            for jj in range(GJ):
                                jt = g * GJ + jj
                                nc.tensor.matmul(
                                    oT[h][:],
                                    h_all[:, (jt * nheads + h) * 65:
                                          (jt * nheads + h) * 65 + 65],
                                    pg[:, jj * IB:(jj + 1) * IB],
                                    start=(jt == 0), stop=(jt == NT - 1))
                            if lag is not None and g == POOLG + 3:
                                lg, lpg = lag
                                lag = None
                                for jj in range(GJ):
                                    jt = lg * GJ + jj
                                    nc.tensor.matmul(
                                        oT[h][:],
                                        h_all[:, (jt * nheads + h) * 65:
                                              (jt * nheads + h) * 65 + 65],
                                        lpg[:, jj * IB:(jj + 1) * IB],
                                        start=(jt == 0), stop=(jt == NT - 1))
                            if (on_head_done is not None and h > 0
                                    and g == 1):
                                on_head_done(h - 1, oT[h - 1])
                    if on_head_done is not None:
                        on_head_done(nheads - 1, oT[nheads - 1])
                    return oT

                def normalize(oTh, h, nheads, lname):
                    """softmax-normalize one head via transposed reciprocal:
                    returns SBUF [64, IB] f32 tile."""
                    dn = wpool.tile([1, IB], F32, tag="nrow", bufs=2,
                                    name=f"dn_{lname}_{h}")
                    nc.scalar.copy(dn[:], oTh[64:65, :])
                    dnT = pp.tile([128, 4], F32, tag="work",
                                  name=f"dnT_{lname}_{h}")
                    for b in range(4):
                        nc.tensor.transpose(dnT[:, b:b + 1],
                                            dn[0:1, b * 128:(b + 1) * 128],
                                            id_sb[0:1, 0:1])
                    rT = wpool.tile([128, 4], F32, tag="rT", bufs=2,
                                    name=f"rT_{lname}_{h}")
                    nc.vector.reciprocal(rT[:], dnT[:])
                    rps = pp.tile([1, IB], F32, tag="work",
                                  name=f"rps_{lname}_{h}")
                    for b in range(4):
                        nc.tensor.transpose(rps[0:1, b * 128:(b + 1) * 128],
                                            rT[:, b:b + 1], id_sb[:, 0:128])
                    rrow = wpool.tile([1, IB], F32R, tag="nrow", bufs=2,
                                      name=f"rrow_{lname}_{h}")
                    nc.scalar.copy(rrow[:], rps[:])
                    prb = pp.tile([O, IB], F32, tag="work",
                                  name=f"prb_{lname}_{h}")
                    nc.tensor.matmul(prb[:], ones_c[0:1, 0:O], rrow[:])
                    rb = wpool.tile([O, IB], F32, tag="rb", bufs=1,
                                    name=f"rb_{lname}_{h}")
                    nc.scalar.copy(rb[:], prb[:])
                    z = wpool.tile([O, IB], F32, tag="z", bufs=1,
                                   name=f"z_{lname}_{h}")
                    nc.vector.tensor_mul(z[:], oTh[0:64, :], rb[:])
                    if debug and lname.endswith("t0") and h == 0:
                        t_ = wpool.tile([65, IB], F32, tag="dbg2", bufs=1)
                        nc.vector.tensor_copy(t_[:], oTh[:])
                        _dma(nc, dbg["d_oT0"][:], t_[:])
                        _dma(nc, dbg["d_z0"][:], z[:])
                    if debug and lname.endswith("t1") and h == 0:
                        t_ = wpool.tile([65, IB], F32, tag="dbg2", bufs=1)
                        nc.vector.tensor_copy(t_[:], oTh[:])
                        _dma(nc, dbg["d_oT1"][:], t_[:])
                        _dma(nc, dbg["d_z1"][:], z[:])
                    return z

                def transition(nheads, w_bf, wlr_sb, next_heads, gh_in, gh,
                               ger_in, ger, h_all_n, lname):
                    """Returns (on_head_done, finish). on_head_done feeds
                    per-head normalize+ELU into xTn as heads complete;
                    finish() does scores/features/all-gathers and returns
                    (F_bc, BT, DT) for the next layer."""
                    xTn = [wpool.tile([128, IB], F32R, tag=f"xTn{k}", bufs=1,
                                      name=f"xTn_{lname}_{k}")
                           for k in range(2)]

                    def on_head_done(h, oTh):
                        z = normalize(oTh, h, nheads, lname)
                        kc, hh = divmod(h, 2)
                        tneg = wpool.tile([O, IB], F32, tag="tneg", bufs=1,
                                          name=f"tn_{lname}_{h}")
                        nc.scalar.activation(tneg[:], z[:], AF.Relu,
                                             scale=-1.0)
                        eneg = wpool.tile([O, IB], F32, tag="eneg", bufs=1,
                                          name=f"en_{lname}_{h}")
                        nc.scalar.activation(eneg[:], tneg[:], AF.Exp,
                                             scale=-1.0)
                        rpos = wpool.tile([O, IB], F32, tag="rpos", bufs=1,
                                          name=f"rp_{lname}_{h}")
                        nc.vector.tensor_scalar(rpos[:], z[:], 0.0, -1.0,
                                                ALU.max, ALU.add)
                        nc.vector.tensor_add(
                            xTn[kc][hh * O:(hh + 1) * O, :], eneg[:], rpos[:])

                    def finish():
                        # next-layer el/er rows; gather er FIRST (small --
                        # unblocks next layer's score computation)
                        pelr = pp.tile([2 * next_heads, IB], F32, tag="work",
                                       name=f"pelr_{lname}")
                        for kc in range(2):
                            nc.tensor.matmul(
                                pelr[:],
                                wlr_sb[:, kc * 2 * next_heads:
                                       (kc + 1) * 2 * next_heads],
                                xTn[kc][:],
                                start=(kc == 0), stop=(kc == 1))
                        F_bc = fbc_tiles(next_heads, pelr, lname)
                        er_sb = wpool.tile([2 * next_heads, IB], F32,
                                           tag="er_sb", bufs=1,
                                           name=f"ersb_{lname}")
                        nc.scalar.copy(er_sb[:], pelr[:])
                        _dma(nc, ger_in[:],
                             er_sb[next_heads:2 * next_heads, :])
                        if sim_mode:
                            _dma(nc, ger[0:next_heads, :], ger_in[:])
                        else:
                            nc.gpsimd.collective_compute(
                                "AllGather", ALU.bypass, replica_groups=rg,
                                ins=[ger_in[:]], outs=[ger[:]])

                        # next-layer local features h = xTn @ W; gather
                        xTn_bf = [wpool.tile([128, IB], BF16, tag=f"xTnb{k}",
                                             bufs=2, name=f"xTnb_{lname}_{k}")
                                  for k in range(2)]
                        for k in range(2):
                            nc.scalar.copy(xTn_bf[k][:], xTn[k][:].bitcast(F32))
                        for ic in range(4):
                            phn = pp.tile([128, next_heads * O], F32,
                                          tag="work", name=f"phn_{lname}_{ic}")
                            for h in range(next_heads):
                                for kc in range(2):
                                    nc.tensor.matmul(
                                        phn[:, h * O:(h + 1) * O],
                                        xTn_bf[kc][:, ic * 128:(ic + 1) * 128],
                                        w_bf[:, (kc * next_heads + h) * O:
                                             (kc * next_heads + h) * O + O],
                                        start=(kc == 0), stop=(kc == 1))
                            hl = wpool.tile([128, next_heads * O], BF16,
                                            tag="hl", bufs=3,
                                            name=f"hl_{lname}_{ic}")
                            nc.scalar.copy(hl[:], phn[:])
                            _dma(nc, gh_in[ic * 128:(ic + 1) * 128, :], hl[:])
                        if sim_mode:
                            _dma(nc, gh[0:IB, :], gh_in[:])
                        else:
                            nc.gpsimd.collective_compute(
                                "AllGather", ALU.bypass, replica_groups=rg,
                                ins=[gh_in[:]], outs=[gh[:]])

                        # gathered er rows -> transposed B/D scalar tables
                        M = NC * next_heads
                        ger_sb = wpool.tile([M, IB], F32, tag="ger_sb", bufs=1,
                                            name=f"gersb_{lname}")
                        _dma(nc, ger_sb[:], ger[:])
                        gT = pp.tile([128, 4 * M], F32, tag="erT", bufs=1,
                                     name=f"gT_{lname}")
                        for b in range(4):
                            nc.tensor.transpose(
                                gT[:, b * M:(b + 1) * M],
                                ger_sb[:, b * 128:(b + 1) * 128],
                                id_sb[0:M, 0:M])
                        gTv = gT[:].rearrange("p (b c x) -> p x c b", b=4,
                                              x=next_heads)
                        BT, DT = bd_tiles(next_heads, lname)
                        for h in range(next_heads):
                            src = gTv[:, h]
                            nc.scalar.activation(
                                BT[h][:].rearrange("p (c b) -> p c b", b=4),
                                src, AF.Exp)
                            nc.scalar.activation(
                                DT[h][:].rearrange("p (c b) -> p c b", b=4),
                                src, AF.Exp, scale=0.2)

                        # gathered h -> per-j-tile [h | ones-column] tiles
                        hv = h_all_n[:].rearrange("p (t h c) -> p t h c",
                                                  t=NT, h=next_heads)
                        hones = h_all_n[:].rearrange("p (q c) -> p q c", c=65)
                        nc.gpsimd.memset(hones[:, :, O:65], 1.0)
                        for jt in range(NT):
                            _dma(nc, hv[:, jt, :, 0:O],
                                 gh[jt * 128:(jt + 1) * 128, :].rearrange(
                                     "p (h o) -> p h o", h=next_heads))
                        if debug and lname.endswith("t0"):
                            _dma(nc, dbg["d_BT1"][:], BT[0][:])
                            _dma(nc, dbg["d_DT1"][:], DT[0][:])
                            _dma(nc, dbg["d_gersb"][:], ger_sb[:])
                            t1_ = wpool.tile([128, IB], F32, tag="dbg3", bufs=1)
                            nc.vector.tensor_copy(t1_[:], F_bc[0][:])
                            _dma(nc, dbg["d_Fbc1"][:], t1_[:])
                            t2_ = wpool.tile([128, IB], F32, tag="dbg5", bufs=1)
                            nc.vector.tensor_copy(t2_[:], xTn[0][:].bitcast(F32))
                            _dma(nc, dbg["d_xTn0"][:], t2_[:])
                            t3_ = wpool.tile([128, H * 65], F32, tag="dbg4", bufs=1)
                            nc.vector.tensor_copy(t3_[:], h_all_n[:, 0:H * 65])
                            _dma(nc, dbg["d_hall1"][:], t3_[:])
                        return F_bc, BT, DT

                    return on_head_done, finish

                # ================= layer 0 =================
                h_all1 = wpool.tile([128, NT * H * 65], BF16, tag="h_all",
                                    bufs=2)
                ohd0, fin0 = transition(H, w1_bf, wlr1_r, H, gh1_in, gh1,
                                        ger1_in, ger1, h_all1, f"r{rep}t0")
                attention(H, h_all0, F_bc0, BT0, DT0, f"r{rep}l0",
                          on_head_done=ohd0)
                F_bc1, BT1, DT1 = fin0()

                # ================= layer 1 =================
                h_all2 = wpool.tile([128, NT * 65], BF16, tag="h_all", bufs=2)
                ohd1, fin1 = transition(H, w2_bf, wlr2_r, 1, gh2_in, gh2,
                                        ger2_in, ger2, h_all2, f"r{rep}t1")
                attention(H, h_all1, F_bc1, BT1, DT1, f"r{rep}l1",
                          on_head_done=ohd1)
                F_bc2, BT2, DT2 = fin1()

                # ================= layer 2 =================
                oT2 = attention(1, h_all2, F_bc2, BT2, DT2, f"r{rep}l2")
                zf = normalize(oT2[0], 0, 1, f"r{rep}l2f")
                _dma(nc, y[:], zf[:])

    nc.compile()
    return nc


def _get_nc():
    if "nc" not in _CACHE:
        _CACHE["nc"] = _build()
    return _CACHE["nc"]


def kernel(x, adj, W0, a0, W1, a1, W2, a2, **_):
    x = np.asarray(x, np.float32)
    adj = np.asarray(adj)
    W0 = np.asarray(W0, np.float32)
    W1 = np.asarray(W1, np.float32)
    W2 = np.asarray(W2, np.float32)
    a0 = np.asarray(a0, np.float32)
    a1 = np.asarray(a1, np.float32)
    a2 = np.asarray(a2, np.float32)

    # host-side layout prep (no model math beyond folding W @ a)
    xTf = np.ascontiguousarray(x.T)
    adj_bf = (adj != 0).astype(ml_dtypes.bfloat16)

    def fold(W, a):
        o = W.shape[-1]
        wl = np.einsum("hdo,ho->dh", W, a[:, :o, 0])
        wr = np.einsum("hdo,ho->dh", W, a[:, o:, 0])
        return np.ascontiguousarray(
            np.concatenate([wl, wr], axis=1).astype(np.float32))

    common = {
        "xTf": xTf,
        "w0": W0, "w1": W1, "w2": W2,
        "wlr0": fold(W0, a0), "wlr1": fold(W1, a1), "wlr2": fold(W2, a2),
    }
    in_maps = []
    for d in range(NC):
        rows = slice(d * IB, (d + 1) * IB)
        maskT = np.ascontiguousarray(adj_bf[rows].T).reshape(NT, 128, IB)
        in_maps.append({
            **common,
            "xT0": np.ascontiguousarray(xTf[:, rows]),
            "maskT": maskT,
        })

    nc = _get_nc()
    _CACHE["in_maps"] = in_maps
    res = run_bass_kernel_spmd(nc, in_maps, core_ids=list(range(NC)))
    out = np.empty((N, O), np.float32)
    for d in range(NC):
        out[d * IB:(d + 1) * IB] = res.results[d]["y"].T
    return out


# revision 30
# speedup vs baseline: 4.5606x; 4.5606x over previous
"""3-layer GAT (graph attention network) on 8 Trainium2 NeuronCores.

Problem: N=4096 nodes, dense adjacency [N,N], 3 GAT layers
  (128 -> 4x64, 256 -> 4x64, 256 -> 1x64), LeakyReLU(0.2) attention,
  masked softmax, ELU between layers.

Sharding: 1D row partition of the attention matrix. Each core owns
IB=512 rows i (queries). Scores/softmax/aggregation for those rows are
computed locally in transposed layout P[j, i] (j on partitions) so the
aggregation matmul contracts j on the partition axis with no transposes.

Score trick (the big win vs a direct port): with e = el_i + er_j,
  exp(leakyrelu(e)) = max(exp(e), exp(0.2 e))
                    = exp(el_i) * max(exp(er_j), exp(-0.8 el_i)*exp(0.2 er_j))
and the exp(el_i) row factor cancels in the softmax, so the masked
unnormalized weight is
  p'[j, i] = mask * max(B_j, F_i * D_j),
  B = exp(er), D = exp(0.2 er), F = exp(-0.8 el).
B/D live as per-partition scalars ([128, NT] transposed via PE), F as a
broadcast tile [128, IB]. Each j-tile then costs ONE fused VectorE
tensor_scalar (bf16 4x: t = max(F*D_j, B_j)) plus a share of ONE batched
mask tensor_tensor (bf16 2x). No per-element ScalarE work and no PE
score matmuls at all.

Inner loop per (head, j-tile):
  t = max(F*D_j, B_j)        -- VectorE tensor_scalar, bf16 4x
  p = t * adjT mask          -- VectorE tensor_mul, bf16 2x, batched x4
  oT[o,i] += [h|1]^T @ p     -- TensorE, accumulating; row 64 = denom

Softmax denominators are reciprocated in transposed layout ([128, 4*H]
after PE transposes) so the DVE divide iterates over 4 elements/lane
instead of 512.

kernel(**inputs) takes the full unsharded inputs and returns the full
[4096, 64] output.
"""

import numpy as np
import ml_dtypes

import concourse.bass as bass
import concourse.mybir as mybir
import concourse.tile as tile
from concourse import bacc
from concourse.bass_utils import run_bass_kernel_spmd

F32 = mybir.dt.float32
BF16 = mybir.dt.bfloat16
F32R = mybir.dt.float32r
AF = mybir.ActivationFunctionType
ALU = mybir.AluOpType

NC = 8          # cores
N = 4096        # nodes
NT = N // 128   # 32 j-tiles
GJ = 4          # j-tiles per mask-multiply batch
IB = N // NC    # 512 rows per core
H = 4           # heads (layers 0,1)
O = 64          # per-head output dim
D0 = 128        # layer-0 input dim
D1 = H * O      # 256, layer-1/2 input dim
ALPHA = 0.2

_CACHE = {}


def _dma(nc, out, in_):
    nc.sync.dma_start(out=out, in_=in_)


def _build(sim_mode=False, reps=1, debug=False):
    nc = bacc.Bacc(None, target_bir_lowering=False,
                   num_devices=1 if sim_mode else NC)

    xTf = nc.dram_tensor("xTf", [D0, N], F32, kind="ExternalInput")
    xT0 = nc.dram_tensor("xT0", [D0, IB], F32, kind="ExternalInput")
    maskT = nc.dram_tensor("maskT", [NT, 128, IB], BF16, kind="ExternalInput")
    w0 = nc.dram_tensor("w0", [H, D0, O], F32, kind="ExternalInput")
    w1 = nc.dram_tensor("w1", [H, D1, O], F32, kind="ExternalInput")
    w2 = nc.dram_tensor("w2", [1, D1, O], F32, kind="ExternalInput")
    wlr0 = nc.dram_tensor("wlr0", [D0, 2 * H], F32, kind="ExternalInput")
    wlr1 = nc.dram_tensor("wlr1", [D1, 2 * H], F32, kind="ExternalInput")
    wlr2 = nc.dram_tensor("wlr2", [D1, 2], F32, kind="ExternalInput")
    y = nc.dram_tensor("y", [O, IB], F32, kind="ExternalOutput")
    if debug:
        dbg = {
            "d_BT0": nc.dram_tensor("d_BT0", [128, NT], F32, kind="ExternalOutput"),
            "d_DT0": nc.dram_tensor("d_DT0", [128, NT], F32, kind="ExternalOutput"),
            "d_Fbc0": nc.dram_tensor("d_Fbc0", [128, IB], F32, kind="ExternalOutput"),
            "d_pg00": nc.dram_tensor("d_pg00", [128, GJ * IB], F32, kind="ExternalOutput"),
            "d_oT0": nc.dram_tensor("d_oT0", [65, IB], F32, kind="ExternalOutput"),
            "d_z0": nc.dram_tensor("d_z0", [O, IB], F32, kind="ExternalOutput"),
            "d_hall0": nc.dram_tensor("d_hall0", [128, H * 65], F32, kind="ExternalOutput"),
            "d_BT1": nc.dram_tensor("d_BT1", [128, NT], F32, kind="ExternalOutput"),
            "d_DT1": nc.dram_tensor("d_DT1", [128, NT], F32, kind="ExternalOutput"),
            "d_Fbc1": nc.dram_tensor("d_Fbc1", [128, IB], F32, kind="ExternalOutput"),
            "d_xTn0": nc.dram_tensor("d_xTn0", [128, IB], F32, kind="ExternalOutput"),
            "d_hall1": nc.dram_tensor("d_hall1", [128, H * 65], F32, kind="ExternalOutput"),
            "d_oT1": nc.dram_tensor("d_oT1", [65, IB], F32, kind="ExternalOutput"),
            "d_z1": nc.dram_tensor("d_z1", [O, IB], F32, kind="ExternalOutput"),
            "d_gersb": nc.dram_tensor("d_gersb", [NC * H, IB], F32, kind="ExternalOutput"),
        }

    rg = [list(range(NC))]

    with tile.TileContext(nc) as tc:
        with (
            tc.tile_pool(name="const", bufs=1) as cpool,
            tc.tile_pool(name="work", bufs=2) as wpool,
            tc.tile_pool(name="psum", bufs=2, space="PSUM") as pp,
            tc.tile_pool(name="dram", bufs=1, space="DRAM") as dpool,
        ):
            # ---------- constants / resident tiles ----------
            mask_sb = cpool.tile([128, NT * IB], BF16)
            mv = mask_sb[:].rearrange("p (t i) -> p t i", t=NT)
            for g in range(8):
                _dma(nc, mv[:, g * 4:(g + 1) * 4, :],
                     maskT[g * 4:(g + 1) * 4].rearrange("t p i -> p t i"))

            xTf_sb = cpool.tile([D0, N], F32)
            _dma(nc, xTf_sb[:], xTf[:])
            xT0_sb = cpool.tile([D0, IB], F32)
            _dma(nc, xT0_sb[:], xT0[:])

            w0_sb = cpool.tile([D0, H * O], F32)
            _dma(nc, w0_sb[:].rearrange("d (h o) -> d h o", h=H),
                 w0[:].rearrange("h d o -> d h o"))
            w1_sb = cpool.tile([128, 2 * H * O], F32)  # [kc] chunks side by side
            w1v = w1_sb[:].rearrange("d (k h o) -> d k h o", k=2, h=H)
            w1s = w1[:].rearrange("h (k d) o -> k d h o", k=2)
            for kc in range(2):
                _dma(nc, w1v[:, kc], w1s[kc])
            w2_sb = cpool.tile([128, 2 * O], F32)
            w2v = w2_sb[:].rearrange("d (k h o) -> d k h o", k=2, h=1)
            w2s = w2[:].rearrange("h (k d) o -> k d h o", k=2)
            for kc in range(2):
                _dma(nc, w2v[:, kc], w2s[kc])

            wlr0_sb = cpool.tile([D0, 2 * H], F32)
            _dma(nc, wlr0_sb[:], wlr0[:])
            wlr1_sb = cpool.tile([128, 2 * 2 * H], F32)
            wlr1v = wlr1_sb[:].rearrange("d (k c) -> d k c", k=2)
            _dma(nc, wlr1v, wlr1[:].rearrange("(k d) c -> d k c", k=2))
            wlr2_sb = cpool.tile([128, 2 * 2], F32)
            wlr2v = wlr2_sb[:].rearrange("d (k c) -> d k c", k=2)
            _dma(nc, wlr2v, wlr2[:].rearrange("(k d) c -> d k c", k=2))

            xTf_bf = cpool.tile([D0, N], BF16)
            nc.vector.tensor_copy(xTf_bf[:], xTf_sb[:])
            w0_bf = cpool.tile([D0, H * O], BF16)
            nc.vector.tensor_copy(w0_bf[:], w0_sb[:])
            w1_bf = cpool.tile([128, 2 * H * O], BF16)
            nc.vector.tensor_copy(w1_bf[:], w1_sb[:])
            w2_bf = cpool.tile([128, 2 * O], BF16)
            nc.vector.tensor_copy(w2_bf[:], w2_sb[:])

            # fp32r-rounded copies for the next-layer score-term matmuls
            # (the BIR verifier requires f32r matmul operands to be
            # produced rounded; layer-0 uses plain f32 matmuls instead)
            wlr1_r = cpool.tile([128, 2 * 2 * H], F32R)
            nc.vector.tensor_copy(wlr1_r[:], wlr1_sb[:])
            wlr2_r = cpool.tile([128, 2 * 2], F32R)
            nc.vector.tensor_copy(wlr2_r[:], wlr2_sb[:])

            ones_c = cpool.tile([1, IB], F32R)
            nc.vector.memset(ones_c[:].bitcast(F32), 1.0)
            ones128 = cpool.tile([128, 128], F32)
            nc.vector.memset(ones128[:], 1.0)
            id_sb = cpool.tile([128, 128], F32)
            nc.gpsimd.affine_select(id_sb[:], ones128[:], [[-1, 128]],
                                    ALU.is_equal, 0.0, base=0,
                                    channel_multiplier=1)

            def bd_tiles(nheads, lname):
                BT = [wpool.tile([128, NT], F32, tag=f"BT{h}", bufs=2,
                                 name=f"BT_{lname}_{h}") for h in range(nheads)]
                DT = [wpool.tile([128, NT], F32, tag=f"DT{h}", bufs=2,
                                 name=f"DT_{lname}_{h}") for h in range(nheads)]
                return BT, DT

            def fbc_tiles(nheads, pelr, lname):
                """rows 0:nheads of pelr (el) -> per-head F = exp(-0.8 el)
                broadcast tiles [128, IB] bf16 via ones-outer-product.
                (engine APs must start at partition 0/32/64, so the per-head
                rows are staged to partition 0 with tiny SBUF DMAs)"""
                F_all = wpool.tile([nheads, IB], F32R, tag="Fall", bufs=1,
                                   name=f"Fall_{lname}")
                nc.scalar.activation(F_all[:], pelr[0:nheads, :], AF.Exp,
                                     scale=-0.8)
                F_bc = []
                for h in range(nheads):
                    r = wpool.tile([1, IB], F32R, tag="Frow", bufs=2,
                                   name=f"Frow_{lname}_{h}")
                    _dma(nc, r[:], F_all[h:h + 1, :])
                    fb = pp.tile([128, IB], F32, tag="work",
                                 name=f"fb_{lname}_{h}")
                    nc.tensor.matmul(fb[:], ones_c[0:1, 0:128], r[0:1, :])
                    t = wpool.tile([128, IB], BF16, tag=f"Fbc{h}", bufs=1,
                                   name=f"Fbc_{lname}_{h}")
                    nc.scalar.copy(t[:], fb[:])
                    F_bc.append(t)
                return F_bc

            for rep in range(reps):
                # ---------- DRAM bounce buffers for collectives ----------
                gh1_in = dpool.tile([IB, D1], BF16)
                gh1 = dpool.tile([N, D1], BF16, addr_space="Shared")
                ger1_in = dpool.tile([H, IB], F32)
                ger1 = dpool.tile([NC * H, IB], F32, addr_space="Shared")
                gh2_in = dpool.tile([IB, O], BF16)
                gh2 = dpool.tile([N, O], BF16, addr_space="Shared")
                ger2_in = dpool.tile([1, IB], F32)
                ger2 = dpool.tile([NC, IB], F32, addr_space="Shared")

                # ================= layer 0 prep =================
                # er0 for ALL nodes, directly transposed: per j-block
                # erT0[:, t*2H:(t+1)*2H] = x[t-block]^T @ wlr0  ([128, 2H])
                erT0 = pp.tile([128, NT * 2 * H], F32, tag="erT", bufs=1,
                               name=f"erT0_{rep}")
                for t in range(NT):
                    nc.tensor.matmul(
                        erT0[:, t * 2 * H:(t + 1) * 2 * H],
                        xTf_sb[:, t * 128:(t + 1) * 128],
                        wlr0_sb[:])
                erT0v = erT0[:].rearrange("p (t c) -> p c t", c=2 * H)
                BT0, DT0 = bd_tiles(H, f"r{rep}l0")
                for h in range(H):
                    nc.scalar.activation(BT0[h][:], erT0v[:, H + h], AF.Exp)
                    nc.scalar.activation(DT0[h][:], erT0v[:, H + h], AF.Exp,
                                         scale=0.2)
                # el0 (local rows) -> F rows -> broadcast tiles
                pl0 = pp.tile([2 * H, IB], F32, tag="work", name=f"pl0_{rep}")
                nc.tensor.matmul(pl0[:], wlr0_sb[:], xT0_sb[:])
                F_bc0 = fbc_tiles(H, pl0, f"r{rep}l0")

                if debug:
                    _dma(nc, dbg["d_BT0"][:], BT0[0][:])
                    _dma(nc, dbg["d_DT0"][:], DT0[0][:])
                    tf_ = wpool.tile([128, IB], F32, tag="dbg3", bufs=1)
                    nc.vector.tensor_copy(tf_[:], F_bc0[0][:])
                    _dma(nc, dbg["d_Fbc0"][:], tf_[:])

                # full h0 (redundantly per core) -> h_all0 [128, NT*(H*65)]
                h_all0 = wpool.tile([128, NT * H * 65], BF16, tag="h_all",
                                    bufs=2)
                h0v = h_all0[:].rearrange("p (t h c) -> p t h c", t=NT, h=H)
                h0ones = h_all0[:].rearrange("p (q c) -> p q c", c=65)
                nc.vector.memset(h0ones[:, :, O:65], 1.0)
                for jt in range(NT):
                    ph = pp.tile([128, H * O], F32, tag="work",
                                 name=f"ph0_{rep}_{jt}")
                    for h in range(H):
                        nc.tensor.matmul(
                            ph[:, h * O:(h + 1) * O],
                            xTf_bf[:, jt * 128:(jt + 1) * 128],
                            w0_bf[:, h * O:(h + 1) * O])
                    nc.scalar.copy(h0v[:, jt, :, 0:O],
                                   ph[:].rearrange("p (h o) -> p h o", h=H))

                if debug:
                    th_ = wpool.tile([128, H * 65], F32, tag="dbg4", bufs=1)
                    nc.vector.tensor_copy(th_[:], h_all0[:, 0:H * 65])
                    _dma(nc, dbg["d_hall0"][:], th_[:])

                def attention(nheads, h_all, F_bc, BT, DT, lname,
                              on_head_done=None):
                    """Row-block attention; oT[h] [65, IB] PSUM accumulators
                    (row 64 = softmax denominator). on_head_done(h, oT_h) is
                    emitted two score-groups into the NEXT head so the
                    engines never stall on the accumulation tail."""
                    oT = [pp.tile([65, IB], F32, tag=f"oT{h}", bufs=1,
                                  name=f"oT_{lname}_{h}")
                          for h in range(nheads)]
                    POOLG = -1  # GPSIMD offload disabled (measured much slower)
                    for h in range(nheads):
                        lag = None
                        for g in range(NT // GJ):
                            use_pool = (g == POOLG)
                            eng = nc.gpsimd if use_pool else nc.vector
                            sg = wpool.tile([128, GJ * IB], BF16,
                                            tag="sgp" if use_pool else "sg",
                                            bufs=1 if use_pool else 2,
                                            name=f"s_{lname}_{h}_{g}")
                            for jj in range(GJ):
                                jt = g * GJ + jj
                                eng.tensor_scalar(
                                    sg[:, jj * IB:(jj + 1) * IB],
                                    F_bc[h][:],
                                    DT[h][:, jt:jt + 1],
                                    BT[h][:, jt:jt + 1],
                                    ALU.mult, ALU.max)
                            pg = wpool.tile([128, GJ * IB], BF16,
                                            tag="pgp" if use_pool else "pg",
                                            bufs=1 if use_pool else 2,
                                            name=f"p_{lname}_{h}_{g}")
                            eng.tensor_mul(
                                pg[:], sg[:],
                                mask_sb[:, g * GJ * IB:(g + 1) * GJ * IB])
                            if use_pool:
                                # defer this group's aggregation until the
                                # slow GPSIMD op has had time to finish
                                lag = (g, pg)
                                continue
                            if debug and lname.endswith("l0") and h == 0 and g == 0:
                                t_ = wpool.tile([128, GJ * IB], F32, tag="dbg1", bufs=1)
                                nc.vector.tensor_copy(t_[:], pg[:])
                                _dma(nc, dbg["d_pg00"][:], t_[:])
                            for jj in range(GJ):
                                jt = g * GJ + jj
                                nc.tensor.matmul(
                                    oT[h][:],
                                    h_all[:, (jt * nheads + h) * 65:
                                          (jt * nheads + h) * 65 + 65],
                                    pg[:, jj * IB:(jj + 1) * IB],
                                    start=(jt == 0), stop=(jt == NT - 1))
                            if lag is not None and g == POOLG + 3:
                                lg, lpg = lag
                                lag = None
                                for jj in range(GJ):
                                    jt = lg * GJ + jj
                                    nc.tensor.matmul(
                                        oT[h][:],
                                        h_all[:, (jt * nheads + h) * 65:
                                              (jt * nheads + h) * 65 + 65],
                                        lpg[:, jj * IB:(jj + 1) * IB],
                                        start=(jt == 0), stop=(jt == NT - 1))
                            if (on_head_done is not None and h > 0
                                    and g == 1):
                                on_head_done(h - 1, oT[h - 1])
                    if on_head_done is not None:
                        on_head_done(nheads - 1, oT[nheads - 1])
                    return oT

                def normalize(oTh, h, nheads, lname):
                    """softmax-normalize one head via transposed reciprocal:
                    returns SBUF [64, IB] f32 tile."""
                    dn = wpool.tile([1, IB], F32, tag="nrow", bufs=2,
                                    name=f"dn_{lname}_{h}")
                    nc.scalar.copy(dn[:], oTh[64:65, :])
                    dnT = pp.tile([128, 4], F32, tag="work",
                                  name=f"dnT_{lname}_{h}")
                    for b in range(4):
                        nc.tensor.transpose(dnT[:, b:b + 1],
                                            dn[0:1, b * 128:(b + 1) * 128],
                                            id_sb[0:1, 0:1])
                    rT = wpool.tile([128, 4], F32, tag="rT", bufs=2,
                                    name=f"rT_{lname}_{h}")
                    nc.vector.reciprocal(rT[:], dnT[:])
                    rps = pp.tile([1, IB], F32, tag="work",
                                  name=f"rps_{lname}_{h}")
                    for b in range(4):
                        nc.tensor.transpose(rps[0:1, b * 128:(b + 1) * 128],
                                            rT[:, b:b + 1], id_sb[:, 0:128])
                    rrow = wpool.tile([1, IB], F32R, tag="nrow", bufs=2,
                                      name=f"rrow_{lname}_{h}")
                    nc.scalar.copy(rrow[:], rps[:])
                    prb = pp.tile([O, IB], F32, tag="work",
                                  name=f"prb_{lname}_{h}")
                    nc.tensor.matmul(prb[:], ones_c[0:1, 0:O], rrow[:])
                    rb = wpool.tile([O, IB], F32, tag="rb", bufs=1,
                                    name=f"rb_{lname}_{h}")
                    nc.scalar.copy(rb[:], prb[:])
                    z = wpool.tile([O, IB], F32, tag="z", bufs=1,
                                   name=f"z_{lname}_{h}")
                    nc.vector.tensor_mul(z[:], oTh[0:64, :], rb[:])
                    if debug and lname.endswith("t0") and h == 0:
                        t_ = wpool.tile([65, IB], F32, tag="dbg2", bufs=1)
                        nc.vector.tensor_copy(t_[:], oTh[:])
                        _dma(nc, dbg["d_oT0"][:], t_[:])
                        _dma(nc, dbg["d_z0"][:], z[:])
                    if debug and lname.endswith("t1") and h == 0:
                        t_ = wpool.tile([65, IB], F32, tag="dbg2", bufs=1)
                        nc.vector.tensor_copy(t_[:], oTh[:])
                        _dma(nc, dbg["d_oT1"][:], t_[:])
                        _dma(nc, dbg["d_z1"][:], z[:])
                    return z

                def transition(nheads, w_bf, wlr_sb, next_heads, gh_in, gh,
                               ger_in, ger, h_all_n, lname):
                    """Returns (on_head_done, finish). on_head_done feeds
                    per-head normalize+ELU into xTn as heads complete;
                    finish() does scores/features/all-gathers and returns
                    (F_bc, BT, DT) for the next layer."""
                    xTn = [wpool.tile([128, IB], F32R, tag=f"xTn{k}", bufs=1,
                                      name=f"xTn_{lname}_{k}")
                           for k in range(2)]

                    def on_head_done(h, oTh):
                        z = normalize(oTh, h, nheads, lname)
                        kc, hh = divmod(h, 2)
                        tneg = wpool.tile([O, IB], F32, tag="tneg", bufs=1,
                                          name=f"tn_{lname}_{h}")
                        nc.scalar.activation(tneg[:], z[:], AF.Relu,
                                             scale=-1.0)
                        eneg = wpool.tile([O, IB], F32, tag="eneg", bufs=1,
                                          name=f"en_{lname}_{h}")
                        nc.scalar.activation(eneg[:], tneg[:], AF.Exp,
                                             scale=-1.0)
                        rpos = wpool.tile([O, IB], F32, tag="rpos", bufs=1,
                                          name=f"rp_{lname}_{h}")
                        nc.vector.tensor_scalar(rpos[:], z[:], 0.0, -1.0,
                                                ALU.max, ALU.add)
                        nc.vector.tensor_add(
                            xTn[kc][hh * O:(hh + 1) * O, :], eneg[:], rpos[:])

                    def finish():
                        # next-layer el/er rows; gather er FIRST (small --
                        # unblocks next layer's score computation)
                        pelr = pp.tile([2 * next_heads, IB], F32, tag="work",
                                       name=f"pelr_{lname}")
                        for kc in range(2):
                            nc.tensor.matmul(
                                pelr[:],
                                wlr_sb[:, kc * 2 * next_heads:
                                       (kc + 1) * 2 * next_heads],
                                xTn[kc][:],
                                start=(kc == 0), stop=(kc == 1))
                        F_bc = fbc_tiles(next_heads, pelr, lname)
                        er_sb = wpool.tile([2 * next_heads, IB], F32,
                                           tag="er_sb", bufs=1,
                                           name=f"ersb_{lname}")
                        nc.scalar.copy(er_sb[:], pelr[:])
                        _dma(nc, ger_in[:],
                             er_sb[next_heads:2 * next_heads, :])
                        if sim_mode:
                            _dma(nc, ger[0:next_heads, :], ger_in[:])
                        else:
                            nc.gpsimd.collective_compute(
                                "AllGather", ALU.bypass, replica_groups=rg,
                                ins=[ger_in[:]], outs=[ger[:]])

                        # next-layer local features h = xTn @ W; gather
                        xTn_bf = [wpool.tile([128, IB], BF16, tag=f"xTnb{k}",
                                             bufs=2, name=f"xTnb_{lname}_{k}")
                                  for k in range(2)]
                        for k in range(2):
                            nc.scalar.copy(xTn_bf[k][:], xTn[k][:].bitcast(F32))
                        for ic in range(4):
                            phn = pp.tile([128, next_heads * O], F32,
                                          tag="work", name=f"phn_{lname}_{ic}")
                            for h in range(next_heads):
                                for kc in range(2):
                                    nc.tensor.matmul(
                                        phn[:, h * O:(h + 1) * O],
                                        xTn_bf[kc][:, ic * 128:(ic + 1) * 128],
                                        w_bf[:, (kc * next_heads + h) * O:
                                             (kc * next_heads + h) * O + O],
                                        start=(kc == 0), stop=(kc == 1))
                            hl = wpool.tile([128, next_heads * O], BF16,
                                            tag="hl", bufs=3,
                                            name=f"hl_{lname}_{ic}")
                            nc.scalar.copy(hl[:], phn[:])
                            _dma(nc, gh_in[ic * 128:(ic + 1) * 128, :], hl[:])
                        if sim_mode:
                            _dma(nc, gh[0:IB, :], gh_in[:])
                        else:
                            nc.gpsimd.collective_compute(
                                "AllGather", ALU.bypass, replica_groups=rg,
                                ins=[gh_in[:]], outs=[gh[:]])

                        # gathered er rows -> transposed B/D scalar tables
                        M = NC * next_heads
                        ger_sb = wpool.tile([M, IB], F32, tag="ger_sb", bufs=1,
                                            name=f"gersb_{lname}")
                        _dma(nc, ger_sb[:], ger[:])
                        gT = pp.tile([128, 4 * M], F32, tag="erT", bufs=1,
                                     name=f"gT_{lname}")
                        for b in range(4):
                            nc.tensor.transpose(
                                gT[:, b * M:(b + 1) * M],
                                ger_sb[:, b * 128:(b + 1) * 128],
                                id_sb[0:M, 0:M])
                        gTv = gT[:].rearrange("p (b c x) -> p x c b", b=4,
                                              x=next_heads)
                        BT, DT = bd_tiles(next_heads, lname)
                        for h in range(next_heads):
                            src = gTv[:, h]
                            nc.scalar.activation(
                                BT[h][:].rearrange("p (c b) -> p c b", b=4),
                                src, AF.Exp)
                            nc.scalar.activation(
                                DT[h][:].rearrange("p (c b) -> p c b", b=4),
                                src, AF.Exp, scale=0.2)

                        # gathered h -> per-j-tile [h | ones-column] tiles
                        hv = h_all_n[:].rearrange("p (t h c) -> p t h c",
                                                  t=NT, h=next_heads)
                        hones = h_all_n[:].rearrange("p (q c) -> p q c", c=65)
                        nc.vector.memset(hones[:, :, O:65], 1.0)
                        for jt in range(NT):
                            _dma(nc, hv[:, jt, :, 0:O],
                                 gh[jt * 128:(jt + 1) * 128, :].rearrange(
                                     "p (h o) -> p h o", h=next_heads))
                        if debug and lname.endswith("t0"):
                            _dma(nc, dbg["d_BT1"][:], BT[0][:])
                            _dma(nc, dbg["d_DT1"][:], DT[0][:])
                            _dma(nc, dbg["d_gersb"][:], ger_sb[:])
                            t1_ = wpool.tile([128, IB], F32, tag="dbg3", bufs=1)
                            nc.vector.tensor_copy(t1_[:], F_bc[0][:])
                            _dma(nc, dbg["d_Fbc1"][:], t1_[:])
                            t2_ = wpool.tile([128, IB], F32, tag="dbg5", bufs=1)
                            nc.vector.tensor_copy(t2_[:], xTn[0][:].bitcast(F32))
                            _dma(nc, dbg["d_xTn0"][:], t2_[:])
                            t3_ = wpool.tile([128, H * 65], F32, tag="dbg4", bufs=1)
                            nc.vector.tensor_copy(t3_[:], h_all_n[:, 0:H * 65])
                            _dma(nc, dbg["d_hall1"][:], t3_[:])
                        return F_bc, BT, DT

                    return on_head_done, finish

                # ================= layer 0 =================
                h_all1 = wpool.tile([128, NT * H * 65], BF16, tag="h_all",
                                    bufs=2)
                ohd0, fin0 = transition(H, w1_bf, wlr1_r, H, gh1_in, gh1,
                                        ger1_in, ger1, h_all1, f"r{rep}t0")
                attention(H, h_all0, F_bc0, BT0, DT0, f"r{rep}l0",
                          on_head_done=ohd0)
                F_bc1, BT1, DT1 = fin0()

                # ================= layer 1 =================
                h_all2 = wpool.tile([128, NT * 65], BF16, tag="h_all", bufs=2)
                ohd1, fin1 = transition(H, w2_bf, wlr2_r, 1, gh2_in, gh2,
                                        ger2_in, ger2, h_all2, f"r{rep}t1")
                attention(H, h_all1, F_bc1, BT1, DT1, f"r{rep}l1",
                          on_head_done=ohd1)
                F_bc2, BT2, DT2 = fin1()

                # ================= layer 2 =================
                oT2 = attention(1, h_all2, F_bc2, BT2, DT2, f"r{rep}l2")
                zf = normalize(oT2[0], 0, 1, f"r{rep}l2f")
                _dma(nc, y[:], zf[:])

    nc.compile()
    return nc


def _get_nc():
    if "nc" not in _CACHE:
        _CACHE["nc"] = _build()
    return _CACHE["nc"]


def kernel(x, adj, W0, a0, W1, a1, W2, a2, **_):
    x = np.asarray(x, np.float32)
    adj = np.asarray(adj)
    W0 = np.asarray(W0, np.float32)
    W1 = np.asarray(W1, np.float32)
    W2 = np.asarray(W2, np.float32)
    a0 = np.asarray(a0, np.float32)
    a1 = np.asarray(a1, np.float32)
    a2 = np.asarray(a2, np.float32)

    # host-side layout prep (no model math beyond folding W @ a)
    xTf = np.ascontiguousarray(x.T)
    adj_bf = (adj != 0).astype(ml_dtypes.bfloat16)

    def fold(W, a):
        o = W.shape[-1]
        wl = np.einsum("hdo,ho->dh", W, a[:, :o, 0])
        wr = np.einsum("hdo,ho->dh", W, a[:, o:, 0])
        return np.ascontiguousarray(
            np.concatenate([wl, wr], axis=1).astype(np.float32))

    common = {
        "xTf": xTf,
        "w0": W0, "w1": W1, "w2": W2,
        "wlr0": fold(W0, a0), "wlr1": fold(W1, a1), "wlr2": fold(W2, a2),
    }
    in_maps = []
    for d in range(NC):
        rows = slice(d * IB, (d + 1) * IB)
        maskT = np.ascontiguousarray(adj_bf[rows].T).reshape(NT, 128, IB)
        in_maps.append({
            **common,
            "xT0": np.ascontiguousarray(xTf[:, rows]),
            "maskT": maskT,
        })

    nc = _get_nc()
    _CACHE["in_maps"] = in_maps
    res = run_bass_kernel_spmd(nc, in_maps, core_ids=list(range(NC)))
    out = np.empty((N, O), np.float32)
    for d in range(NC):
        out[d * IB:(d + 1) * IB] = res.results[d]["y"].T
    return out
